# revision 4
# baseline (speedup 1.0000x reference)
"""Trainium2 Bass kernel for the thin-plate-spline RBF layer.

reference:  out[b,n,d] = sum_m phi(|x_bn - c_bm|) * w[b,m,d],
            phi(r) = r^2 * log(r + 1e-6)

Device algorithm (per core, N sharded 8 ways):
  dist2[m,n] = sum_k a_k[m] * b_k[n]   -- rank-15 bf16 split-precision
      expansion of |x-c|^2 (coordinates centered, split into bf16 hi/lo;
      bf16 products are exact under fp32 PSUM accumulation).  The 64
      (nt, h, b) dist2 matmuls (512 n-cols each) are packed 3-per-PSUM
      tile (1536 cols, 3 banks) with per-batch 32-row strips
      (tile_position row tiling) running concurrently.
  L[m,n] = ln(dist2 + 5e-5)            (ScalarE, one 1536-col ACTIVATE
      per d2 tile -- amortizes the ~352-cycle per-instruction overhead;
      ScalarE is the kernel's bottleneck engine)
  The elementwise dist2*L multiply is eliminated algebraically:
    out[b,n,d] = sum_k b_k[n] * S[(k,b,d), n],
    S = sum_m (0.5 * a_k[m] * w[m,d]) * L[m,n]   (TensorE,
        batch-stacked block-diagonal weights, 60 columns).
  Early junk matmuls interleave with the first dist2 groups to hold the
  PE HAM clock gate open before phase-2 demand arrives; they write a
  dead PSUM tile and are never read.
"""
import sys

sys.path.insert(0, "/opt/trn_rl_repo")

import numpy as np
import ml_dtypes

BF16 = np.dtype(ml_dtypes.bfloat16)

B, M, N, NCORES = 4, 256, 32768, 8
NS = N // NCORES          # 4096 dense points per core
NT = 512                  # n-tile (one PSUM bank of fp32)
NTILES = NS // NT         # 8
HALVES = M // 128         # 2
NBLK = B * HALVES         # 8 contraction blocks of 128
KD = 15                   # dist2 split-precision rank
J = 5 * B * 3             # 60 stacked S columns, j = k*12 + b*3 + d
JP = 64                   # per-block stationary column pitch
DELTA = 5e-5
Q = NTILES * HALVES * B   # 64 dist2 matmuls of NT cols each
GRP = 3                   # dist2 matmuls per PSUM tile (3 banks)
# tile 0 holds q=0 alone (fast first ACT); tiles 1..21 hold 3 each
NGRPS = 1 + (Q - 1) // GRP

_compiled = None


def _q2tile(q):
    """dist2 matmul index -> (L tile index, column slot)."""
    if q == 0:
        return 0, 0
    return (q + 2) // GRP, (q - 1) % GRP


def _build_nc():
    import concourse.bacc as bacc
    import concourse.mybir as mybir
    from concourse.tile import TileContext

    f32 = mybir.dt.float32
    bf = mybir.dt.bfloat16
    f16 = mybir.dt.float16
    nc = bacc.Bacc("TRN2")

    daug_d = nc.dram_tensor("daug", [128, NS], bf, kind="ExternalInput")
    bcs_d = nc.dram_tensor("bcs", [J, NS], f32, kind="ExternalInput")
    cpa_d = nc.dram_tensor("cpa", [128, HALVES * 128], bf, kind="ExternalInput")
    wps_d = nc.dram_tensor("wps", [128, NBLK * JP], f16, kind="ExternalInput")
    rmat_d = nc.dram_tensor("rmat", [J, 12], f16, kind="ExternalInput")
    out_d = nc.dram_tensor("outb", [12, NS], f32, kind="ExternalOutput")

    with TileContext(nc) as tc:
        with (
            tc.tile_pool(name="singles", bufs=1) as singles,
            tc.tile_pool(name="lpool", bufs=10) as lpool,
            tc.tile_pool(name="zpool", bufs=3) as zpool,
            tc.tile_pool(name="d2pool", bufs=2, space="PSUM") as d2pool,
            tc.tile_pool(name="spool", bufs=1, space="PSUM") as spool,
            tc.tile_pool(name="jpool", bufs=1, space="PSUM") as jpool,
        ):
            delta_t = singles.tile([128, 1], f32)
            nc.vector.memset(delta_t, DELTA)
            scratch = singles.tile([128, NT], bf)
            nc.vector.memset(scratch[:], 0.0)

            # input DMAs, spread across the three DGE-capable queues by
            # need-time.  Every DMA has ~2.5us trigger->first-descriptor
            # latency, so first-needed inputs are split fine.
            cpa_t = singles.tile([128, HALVES * 128], bf)
            nc.gpsimd.dma_start(out=cpa_t[:, :128], in_=cpa_d[:, :128])
            nc.gpsimd.dma_start(out=cpa_t[:, 128:], in_=cpa_d[:, 128:])
            wps_t = singles.tile([128, NBLK * JP], f16)
            nc.gpsimd.dma_start(out=wps_t[:], in_=wps_d[:])
            rmat_t = singles.tile([J, 12], f16)
            nc.gpsimd.dma_start(out=rmat_t[:], in_=rmat_d[:])
            daug_t = singles.tile([128, NS], bf)
            nc.sync.dma_start(out=daug_t[:, :NT], in_=daug_d[:, :NT])
            nc.sync.dma_start(out=daug_t[:, NT : 2 * NT],
                              in_=daug_d[:, NT : 2 * NT])
            nc.sync.dma_start(out=daug_t[:, 2 * NT :], in_=daug_d[:, 2 * NT :])
            bcs_t = singles.tile([J, NS], f32)
            nc.scalar.dma_start(out=bcs_t[:, : NS // 2], in_=bcs_d[:, : NS // 2])
            nc.gpsimd.dma_start(out=bcs_t[:, NS // 2 :], in_=bcs_d[:, NS // 2 :])
            out_sb = singles.tile([12, NS], f32)

            # Front-loaded junk matmuls: the scheduler hoists these (no
            # input deps) into the otherwise-dead DMA-latency window, so
            # the PE HAM clock gate is already open when real work lands.
            jt = jpool.tile([128, NT], f32)
            for _ in range(5):
                nc.tensor.matmul(jt[:], scratch[:, :128], scratch[:],
                                 start=True, stop=True)

            # ---- phase 1 emission: dist2 matmuls + ln ----
            # d2 tile g holds GRP consecutive q-slabs; q = ((nt*2+h)*4+b).
            # The three matmuls of a group go to distinct PE row strips
            # (tile_position) and distinct PSUM banks -> concurrent.
            ltiles = []
            q = 0
            for g in range(NGRPS):
                cnt = 1 if g == 0 else min(GRP, Q - q)
                d2 = d2pool.tile([128, GRP * NT], f32, tag="d2")
                for j_ in range(cnt):
                    nt_, h, b = q // 8, (q % 8) // 4, q % 4
                    nc.tensor.matmul(
                        d2[:, j_ * NT : (j_ + 1) * NT],
                        cpa_t[32 * b : 32 * b + KD,
                              h * 128 : (h + 1) * 128],
                        daug_t[32 * b : 32 * b + KD,
                               nt_ * NT : (nt_ + 1) * NT],
                        start=True,
                        stop=True,
                        tile_position=(32 * b, 0),
                    )
                    q += 1
                lt = lpool.tile([128, GRP * NT], f16, tag="L")
                if g == NGRPS - 1:
                    # split the final group's ln into per-slab ACTIVATEs so
                    # the last S chain can start before the whole group is
                    # done -- shortens the post-ln tail
                    for j_ in range(cnt):
                        csl = slice(j_ * NT, (j_ + 1) * NT)
                        nc.scalar.activation(
                            out=lt[:, csl], in_=d2[:, csl],
                            func=mybir.ActivationFunctionType.Ln,
                            bias=delta_t[:], scale=1.0,
                        )
                else:
                    nc.scalar.activation(
                        out=lt[:, : cnt * NT],
                        in_=d2[:, : cnt * NT],
                        func=mybir.ActivationFunctionType.Ln,
                        bias=delta_t[:],
                        scale=1.0,
                    )
                ltiles.append(lt)
                if 1 <= g <= 3:
                    # small filler keeps PE duty high until phase-2 demand
                    nc.tensor.matmul(jt[:, :256], scratch[:, :128],
                                     scratch[:, :256], start=True, stop=True)

            # ---- phase 2 emission: S chains, combine, reduce, store ----
            # s_c and o2 share one PSUM bank: the 12-row o2 output is
            # col-tiled to array columns 64-75 / PSUM partitions 64-75.
            for nt_ in range(NTILES):
                nsl = slice(nt_ * NT, (nt_ + 1) * NT)
                sb_t = spool.tile([128, NT], f32, tag="S")
                s_c = sb_t[0:JP]
                for idx in range(8):
                    h, b = idx // 4, idx % 4
                    l = 2 * b + h
                    t, c = _q2tile(nt_ * 8 + idx)
                    nc.tensor.matmul(
                        s_c[:],
                        wps_t[:, l * JP : (l + 1) * JP],
                        ltiles[t][:, c * NT : (c + 1) * NT],
                        start=(idx == 0),
                        stop=(idx == 7),
                    )
                z_t = zpool.tile([J, NT], f16, tag="z")
                nc.vector.tensor_mul(z_t[:], s_c[0:J, :], bcs_t[:, nsl])
                o2 = sb_t[64 : 64 + 12]
                nc.tensor.matmul(o2[:], rmat_t[:], z_t[:],
                                 start=True, stop=True,
                                 tile_position=(0, 64))
                nc.vector.tensor_copy(out_sb[:, nsl], o2[:])
            # merged output DMAs; the last n-tile ships alone to keep the
            # final store small
            nc.sync.dma_start(out=out_d[:, : 4 * NT], in_=out_sb[:, : 4 * NT])
            nc.sync.dma_start(out=out_d[:, 4 * NT : 7 * NT],
                              in_=out_sb[:, 4 * NT : 7 * NT])
            nc.sync.dma_start(out=out_d[:, 7 * NT :], in_=out_sb[:, 7 * NT :])

    nc.compile()
    return nc


def _split3(v):
    """3-way bf16 split of float64 array."""
    hi = v.astype(BF16)
    r1 = v - hi.astype(np.float64)
    mid = r1.astype(BF16)
    r2 = r1 - mid.astype(np.float64)
    lo = r2.astype(BF16)
    return hi, mid, lo


def _host_prep(sparse_disp, original_cp, original_dense):
    """Build per-core input maps for the device kernel."""
    x = original_dense.astype(np.float64) - 0.5   # (B, N, 3) centered
    c = original_cp.astype(np.float64) - 0.5      # (B, M, 3)
    w = sparse_disp.astype(np.float32)            # (B, M, 3)

    # ---- control-point side (shared by all cores) ----
    p = c.astype(BF16)
    q = (c - p.astype(np.float64)).astype(BF16)
    t_hi, t_mid, t_lo = _split3((c * c).sum(-1))
    ones_m = np.ones((B, M), BF16)

    # per-batch KD rows: [p x3, p x3, q x3, t_hi, t_mid, t_lo, 1, 1, 1]
    cpa_full = np.empty((B, KD, M), BF16)
    for d in range(3):
        cpa_full[:, d, :] = p[:, :, d]
        cpa_full[:, 3 + d, :] = p[:, :, d]
        cpa_full[:, 6 + d, :] = q[:, :, d]
    cpa_full[:, 9, :] = t_hi
    cpa_full[:, 10, :] = t_mid
    cpa_full[:, 11, :] = t_lo
    cpa_full[:, 12, :] = ones_m
    cpa_full[:, 13, :] = ones_m
    cpa_full[:, 14, :] = ones_m

    # stacked stationary: rows 32b..32b+KD, cols h*128..
    cpa = np.zeros((128, HALVES * 128), BF16)
    for b in range(B):
        for h in range(HALVES):
            cpa[32 * b : 32 * b + KD, h * 128 : (h + 1) * 128] = \
                cpa_full[b, :, h * 128 : (h + 1) * 128]

    # wps packed [128, NBLK*64]: block l = 2b+h at cols l*64..
    wps = np.zeros((128, NBLK * JP), np.float32)
    c32 = c.astype(np.float32)
    a5 = np.stack(
        [c32[:, :, 0], c32[:, :, 1], c32[:, :, 2],
         (c32 * c32).sum(-1), np.ones((B, M), np.float32)],
        axis=1,
    )  # (B, 5, M)
    for b in range(B):
        for h in range(HALVES):
            l = 2 * b + h
            msl = slice(h * 128, (h + 1) * 128)
            for k in range(5):
                for d in range(3):
                    j = k * 12 + b * 3 + d
                    wps[:, l * JP + j] = 0.5 * a5[b, k, msl] * w[b, msl, d]

    rmat = np.zeros((J, 12), np.float32)
    for j in range(J):
        rmat[j, j % 12] = 1.0

    # ---- dense-point side (per core) ----
    u_all = x.astype(BF16)
    v_all = (x - u_all.astype(np.float64)).astype(BF16)
    s_all = (x * x).sum(-1)

    in_maps = []
    for core in range(NCORES):
        csl = slice(core * NS, (core + 1) * NS)
        u = u_all[:, csl, :].astype(np.float32)
        v = v_all[:, csl, :].astype(np.float32)
        s_hi, s_mid, s_lo = _split3(s_all[:, csl])
        ones_n = np.ones((B, NS), BF16)

        daug_b = np.empty((B, KD, NS), BF16)
        for d in range(3):
            daug_b[:, d, :] = (-2.0 * u[:, :, d]).astype(BF16)
            daug_b[:, 3 + d, :] = (-2.0 * v[:, :, d]).astype(BF16)
            daug_b[:, 6 + d, :] = (-2.0 * u[:, :, d]).astype(BF16)
        daug_b[:, 9, :] = ones_n
        daug_b[:, 10, :] = ones_n
        daug_b[:, 11, :] = ones_n
        daug_b[:, 12, :] = s_hi
        daug_b[:, 13, :] = s_mid
        daug_b[:, 14, :] = s_lo

        daug = np.zeros((128, NS), BF16)
        for b in range(B):
            daug[32 * b : 32 * b + KD] = daug_b[b]

        xs = x[:, csl, :].astype(np.float32)
        baug5 = np.stack(
            [-2.0 * xs[:, :, 0], -2.0 * xs[:, :, 1], -2.0 * xs[:, :, 2],
             np.ones((B, NS), np.float32), (xs * xs).sum(-1)],
            axis=1,
        )  # (B, 5, NS)
        bc = np.empty((J, NS), np.float32)
        for k in range(5):
            for b in range(B):
                for d in range(3):
                    bc[k * 12 + b * 3 + d] = baug5[b, k]

        in_maps.append(
            {
                "daug": daug,
                "bcs": bc,
                "cpa": cpa,
                "wps": wps.astype(np.float16),
                "rmat": rmat.astype(np.float16),
            }
        )
    return in_maps


def _assemble(results):
    out = np.empty((B, N, 3), np.float32)
    for core, r in enumerate(results):
        o = r["outb"]  # (12, NS) rows b*3+d
        out[:, core * NS : (core + 1) * NS, :] = (
            o.reshape(B, 3, NS).transpose(0, 2, 1)
        )
    return out


def kernel(sparse_disp, original_cp, original_dense):
    global _compiled
    from concourse.bass_utils import run_bass_kernel_spmd

    # force genuine numpy: _host_prep's float64 split-precision math
    # would silently degrade on jax arrays (x64 disabled by default)
    sparse_disp = np.asarray(sparse_disp)
    original_cp = np.asarray(original_cp)
    original_dense = np.asarray(original_dense)

    if _compiled is None:
        _compiled = _build_nc()
    in_maps = _host_prep(sparse_disp, original_cp, original_dense)
    res = run_bass_kernel_spmd(_compiled, in_maps, core_ids=list(range(NCORES)))
    return _assemble(res.results)
